# revision 46
# baseline (speedup 1.0000x reference)
# GAT (2-layer, DGL GATConv mean-path) on 8 TRN2 NeuronCores.
#
# Sharding: nodes partitioned by dst across 8 cores (2500 each); edges live on
# the core owning their dst node. Per core:
#   - node pass (replicated): ft = x @ W1 in bf16 with fused el/er attention
#     columns, written to two gather tables: t1ft (bf16 [N,256] packed rows)
#     and t1e (f32 [N,64] rows holding el|er; f32 rows hit the 256B gather
#     granule at 64 elements instead of 128, halving gather output elements).
#   - layer-1 edge pass: dst-sorted edges in 128-edge chunks; three
#     dma_gathers per 16-chunk batch (ft[src], el[src], er[dst]);
#     exp(leaky_relu(el+er)); segment softmax-sum via a one-hot (S) matmul
#     accumulating numerator AND denominator in PSUM. One-hot build and the
#     ft*exp multiply run in bf16 2x DVE mode (broadcasts ride on dup-pairs
#     so the innermost dim stays packed).
#   - layer 2: per-core compact ft2|el2|er2 rows (bf16 [2560,42]) AllGathered
#     in 5-block groups from the Activation engine (overlapping the remaining
#     layer-1 edge pass), then repacked into a 128-elem-pitch gather table.
#     The layer-2 edge pass gathers only src rows; er2[dst] is broadcast
#     on-chip: transpose the one-hot, then one matmul per chunk against the
#     block's own er2 column.
# exp() without max-subtraction is exact for the softmax ratio (values are
# O(1); reference subtracts segment max only for overflow safety).

import math
import sys
from contextlib import ExitStack

import numpy as np

if "/opt/trn_rl_repo" not in sys.path:
    sys.path.insert(0, "/opt/trn_rl_repo")

import concourse.bass as bass
import concourse.mybir as mybir
import concourse.tile as tile
from concourse import bacc
from concourse.masks import make_identity
from concourse.replica_groups import filter_and_check_groups

F32 = mybir.dt.float32
BF16 = mybir.dt.bfloat16
I16 = mybir.dt.int16
AX = mybir.AxisListType
ALU = mybir.AluOpType
ACTF = mybir.ActivationFunctionType


class Cfg:
    def __init__(self, N=20000, F=128, H1=8, D1=32, C=40, ncores=8, neg=0.2):
        self.N, self.F, self.H1, self.D1, self.C = N, F, H1, D1, C
        self.NCORES, self.NEG = ncores, neg
        self.FT1 = H1 * D1                      # 256
        assert N % ncores == 0
        self.NB = N // ncores                   # 2500 nodes/core
        self.BLK = 128
        self.NBLK = math.ceil(self.NB / self.BLK)   # 20 dst blocks/core
        self.NPAD = self.NBLK * self.BLK            # 2560
        self.CH = 128                                # edges per chunk
        self.GB = 16                                 # chunks / L1 gather batch
        self.GB2 = 16                                # chunks / L2 gather batch
        self.NIDX = self.GB * self.CH                # idx per L1 gather
        self.NIDX2 = self.GB2 * self.CH              # idx per L2 gather
        # L1 tables: t1ft bf16 [N, 256] packed; t1e f32 [N, 64] = [el 8|er 8|0]
        self.ROW1F = self.FT1                        # 256
        self.ROW1E = 64
        self.W1EXT = self.FT1 + 2 * H1               # 272 fused matmul cols
        self.NSUB = 4                    # node tiles per stage-1 super-tile
        self.NT = math.ceil(N / 128)     # node tiles (157)
        # layer 2: compact 42-col bf16 rows AllGathered in CGRP-block groups,
        # then repacked into a 128-elem-pitch local gather table
        self.C2 = C + 2                              # 42
        self.ROW2 = 128
        self.EL2, self.ER2 = C, C + 1
        self.CGRP = self.NBLK            # single end-of-stage-2 collective
        assert self.NBLK % self.CGRP == 0
        self.KT2 = self.FT1 // 128       # K-halves for layer-2 matmul (2)
        assert self.FT1 % 128 == 0


CFG = Cfg()


# ---------------------------------------------------------------- host prep

def _wrap16(a):
    """Edge-index array [E_pad] -> dma_gather idx layout [128, E_pad//16]."""
    cols = np.ascontiguousarray(a.reshape(-1, 16).T).astype(np.int16)
    return np.ascontiguousarray(np.tile(cols, (8, 1)))


def _host_prep(cfg, src, dst):
    """Relabel nodes to balance per-(core,block) edge counts, then
    sort/bucket edges by dst; identical chunk structure on all cores."""
    import heapq
    src = np.asarray(src).astype(np.int64)
    dst = np.asarray(dst).astype(np.int64)
    deg = np.bincount(dst, minlength=cfg.N)
    nbuckets = cfg.NCORES * cfg.NBLK
    lastcap = cfg.NB - (cfg.NBLK - 1) * cfg.BLK
    capn = np.array([lastcap if (i % cfg.NBLK) == cfg.NBLK - 1 else cfg.BLK
                     for i in range(nbuckets)])
    fill = [[] for _ in range(nbuckets)]
    h = [(0, i) for i in range(nbuckets)]
    heapq.heapify(h)
    for n in np.argsort(-deg, kind="stable"):
        while True:
            l, i = heapq.heappop(h)
            if len(fill[i]) < capn[i]:
                fill[i].append(n)
                heapq.heappush(h, (l + int(deg[n]), i))
                break
    order = np.concatenate([np.array(f, np.int64) for f in fill])
    lab = np.empty(cfg.N, np.int64)
    lab[order] = np.arange(cfg.N)
    src = lab[src]
    dst = lab[dst]
    perm = np.argsort(dst, kind="stable")
    ss, dd = src[perm], dst[perm]
    core_of = dd // cfg.NB
    rem = dd % cfg.NB
    blk_of = rem // cfg.BLK
    vloc_of = rem % cfg.BLK

    counts = np.zeros((cfg.NCORES, cfg.NBLK), np.int64)
    np.add.at(counts, (core_of, blk_of), 1)
    chunks_pb = np.maximum(1, -(-counts // cfg.CH)).max(axis=0)     # [NBLK]
    K = int(chunks_pb.sum())
    gq = max(cfg.GB, cfg.GB2)
    K_pad = -(-K // gq) * gq
    cb = np.concatenate(
        [np.repeat(np.arange(cfg.NBLK), chunks_pb),
         np.full(K_pad - K, cfg.NBLK - 1)]).astype(np.int64)
    blk_start_chunk = np.concatenate([[0], np.cumsum(chunks_pb)])
    E_pad = K_pad * cfg.CH

    src_e = np.zeros((cfg.NCORES, E_pad), np.int64)
    dst_e = np.zeros((cfg.NCORES, E_pad), np.int64)
    vloc_e = np.full((cfg.NCORES, E_pad), 999.0, np.float32)
    for c in range(cfg.NCORES):
        mc = core_of == c
        sc, bc, vc, dc = ss[mc], blk_of[mc], vloc_of[mc], dd[mc]
        for b in range(cfg.NBLK):
            mb = bc == b
            cnt = int(mb.sum())
            off = int(blk_start_chunk[b]) * cfg.CH
            src_e[c, off:off + cnt] = sc[mb]
            dst_e[c, off:off + cnt] = dc[mb]
            vloc_e[c, off:off + cnt] = vc[mb].astype(np.float32)

    # layer-2 table rows: collective groups of CGRP blocks;
    # within a group: [core][CGRP*BLK rows]
    gsz = cfg.CGRP * cfg.BLK                   # rows per core per group
    def to_row2(g):
        c = g // cfg.NB
        l = g % cfg.NB
        grp = (l // cfg.BLK) // cfg.CGRP
        return grp * cfg.NCORES * gsz + c * gsz + (l - grp * gsz)

    prep = dict(K_pad=K_pad, cb=cb, order=order)
    prep["src_idx1"] = [_wrap16(src_e[c]) for c in range(cfg.NCORES)]
    prep["dst_idx1"] = [_wrap16(dst_e[c]) for c in range(cfg.NCORES)]
    prep["src_idx2"] = [_wrap16(to_row2(src_e[c])) for c in range(cfg.NCORES)]
    # dst-local one-hot operand, duplicated in pairs so the innermost dim of
    # the is_equal stays packed (bf16 2x DVE mode)
    import ml_dtypes
    dlocd = []
    for c in range(cfg.NCORES):
        v = vloc_e[c].reshape(K_pad, cfg.CH).T            # [128, K_pad]
        vd = np.repeat(v[:, :, None], 2, axis=2).reshape(128, K_pad * 2)
        dlocd.append(np.ascontiguousarray(vd.astype(ml_dtypes.bfloat16)))
    prep["dlocd"] = dlocd
    return prep


# ---------------------------------------------------------------- bass build

def _bcast(ap, axis, n):
    """Insert a broadcast (step-0) axis of size n at `axis`."""
    shape = list(ap.shape)
    shape.insert(axis, n)
    return ap.unsqueeze(axis).broadcast_to(shape)


def _collective_on(eng, kind, op, replica_groups, in_ap, out_ap):
    """Emit an AllGather on `eng` (any engine's sequencer can trigger
    collectives; keeping them off GPSIMD stops them from stalling the
    gather stream)."""
    b = eng.bass
    b.has_collectives = True
    groups = filter_and_check_groups(b.num_devices, replica_groups)
    return eng.add_instruction(
        mybir.InstCollectiveCompute(
            name=f"I-{b.next_id()}",
            kind=kind,
            op=op,
            replica_groups=groups,
            ins=[eng.lower_ap(in_ap)],
            outs=[eng.lower_ap(out_ap)],
            unique_tensors="No",
            cc_dim="Partition",
        )
    )


def _build(cfg, K_pad, cb):
    nc = bacc.Bacc(None, target_bir_lowering=False, num_devices=cfg.NCORES)
    N, F, H1, C, FT1 = cfg.N, cfg.F, cfg.H1, cfg.C, cfg.FT1
    GB, CH, NBLK, BLK = cfg.GB, cfg.CH, cfg.NBLK, cfg.BLK
    W1EXT = cfg.W1EXT

    def din(name, shape, dt=F32):
        return nc.declare_dram_parameter(name, list(shape), dt, isOutput=False)

    xT = din("xT", [F, N])
    w1 = din("W1", [F, FT1])
    alr = din("al_rep", [128, FT1])
    arr = din("ar_rep", [128, FT1])
    w2r = din("W2r", [128, cfg.KT2 * C])
    al2r = din("al2_rep", [128, C])
    ar2r = din("ar2_rep", [128, C])
    iotab = din("iotab", [128, 128], BF16)
    si1 = din("src_idx1", [128, K_pad * 8], I16)
    di1 = din("dst_idx1", [128, K_pad * 8], I16)
    si2 = din("src_idx2", [128, K_pad * 8], I16)
    dlocd = din("dlocd", [128, K_pad * 2], BF16)
    out = nc.declare_dram_parameter("out", [cfg.NPAD, C], F32, isOutput=True)

    t1ft = nc.dram_tensor("t1ft", [N, cfg.ROW1F], BF16)
    t1e = nc.dram_tensor("t1e", [N, cfg.ROW1E], F32)
    table2 = nc.dram_tensor("table2", [cfg.NCORES * cfg.NPAD, cfg.ROW2], BF16)
    t2own_c = nc.dram_tensor("t2own_c", [cfg.NPAD, cfg.C2], BF16)
    table2c = nc.dram_tensor("table2c", [cfg.NCORES * cfg.NPAD, cfg.C2], BF16,
                             addr_space="Shared")

    # chunk -> block bookkeeping (compile-time constant, same on all cores)
    first_of = [k == 0 or cb[k - 1] != cb[k] for k in range(K_pad)]
    last_of = [k == K_pad - 1 or cb[k + 1] != cb[k] for k in range(K_pad)]

    with tile.TileContext(nc) as tc:
        with ExitStack() as ctx:
            # one shared register for the (constant) gather index count
            # (Bacc auto-inserts the 'mlp' GPSIMD library load for dma_gather)
            nidx_reg = nc.gpsimd.to_reg(cfg.NIDX)
            nidx2_reg = nc.gpsimd.to_reg(cfg.NIDX2)
            consts = ctx.enter_context(tc.tile_pool(name="consts", bufs=1))

            sb_w1 = consts.tile([128, FT1], F32)
            sb_alr = consts.tile([128, FT1], F32)
            sb_arr = consts.tile([128, FT1], F32)
            sb_rhs1 = consts.tile([128, W1EXT], F32)
            sb_rhs1b = consts.tile([128, W1EXT], BF16)
            sb_iotab = consts.tile([128, 128], BF16)
            sb_w2r = consts.tile([128, cfg.KT2, C], F32)
            sb_al2 = consts.tile([128, C], F32)
            sb_ar2 = consts.tile([128, C], F32)
            sb_rhs2 = consts.tile([128, cfg.KT2, C + 2], BF16)
            sb_si1 = consts.tile([128, K_pad * 8], I16)
            sb_di1 = consts.tile([128, K_pad * 8], I16)
            sb_si2 = consts.tile([128, K_pad * 8], I16)
            sb_dlocd = consts.tile([128, K_pad, 2], BF16)
            sb_h = consts.tile([128, NBLK, FT1], BF16)
            sb_ident = consts.tile([128, 128], BF16)
            sb_q2 = consts.tile([128, NBLK, 1], BF16)

            nc.sync.dma_start(out=sb_w1, in_=w1[:, :])
            nc.sync.dma_start(out=sb_alr, in_=alr[:, :])
            nc.sync.dma_start(out=sb_arr, in_=arr[:, :])
            nc.sync.dma_start(out=sb_iotab, in_=iotab[:, :])
            nc.sync.dma_start(
                out=sb_w2r, in_=w2r[:, :].rearrange("p (t c) -> p t c", t=cfg.KT2))
            nc.sync.dma_start(out=sb_al2, in_=al2r[:, :])
            nc.sync.dma_start(out=sb_ar2, in_=ar2r[:, :])
            nc.gpsimd.dma_start(out=sb_si1, in_=si1[:, :])
            nc.gpsimd.dma_start(out=sb_di1, in_=di1[:, :])
            nc.gpsimd.dma_start(out=sb_si2, in_=si2[:, :])
            nc.gpsimd.dma_start(
                out=sb_dlocd, in_=dlocd[:, :].rearrange("p (k t) -> p k t", t=2))
            make_identity(nc, sb_ident)

            # zero-fill the never-written pad columns of the gather tables
            # (per-row-bytes DMA accounting makes these near-free)
            zpool = ctx.enter_context(tc.tile_pool(name="zp", bufs=1))
            zt = zpool.tile([128, 64], F32)
            nc.vector.memset(zt, 0.0)
            n_zr1 = N // 128                       # 156 full + remainder
            z1 = bass.AP(tensor=t1e, offset=16,
                         ap=[[cfg.ROW1E * n_zr1, 128], [cfg.ROW1E, n_zr1],
                             [1, 48]])
            nc.scalar.dma_start(
                out=z1, in_=_bcast(zt[:, 0:48], 1, n_zr1))
            zrem = N - n_zr1 * 128
            z1b = bass.AP(tensor=t1e, offset=n_zr1 * 128 * cfg.ROW1E + 16,
                          ap=[[cfg.ROW1E, zrem], [1, 48]])
            nc.scalar.dma_start(out=z1b, in_=zt[0:zrem, 0:48])
            ztb = zpool.tile([128, 128], BF16, tag="ztb")
            nc.vector.memset(ztb, 0.0)
            n_zr2 = cfg.NCORES * cfg.NPAD // 128   # 160
            z2 = bass.AP(tensor=table2, offset=cfg.C2,
                         ap=[[cfg.ROW2 * n_zr2, 128], [cfg.ROW2, n_zr2],
                             [1, cfg.ROW2 - cfg.C2]])
            nc.scalar.dma_start(
                out=z2, in_=_bcast(ztb[:, 0:cfg.ROW2 - cfg.C2], 1, n_zr2))

            # ---- stage 0: fused weights --------------------------------
            s0 = ctx.enter_context(tc.tile_pool(name="s0", bufs=1))
            tmp = s0.tile([128, FT1], F32)
            nc.vector.tensor_mul(tmp, sb_w1, sb_alr)
            nc.vector.tensor_reduce(
                out=sb_rhs1[:, FT1:FT1 + H1],
                in_=tmp.rearrange("p (h d) -> p h d", h=H1),
                axis=AX.X, op=ALU.add)
            tmp2 = s0.tile([128, FT1], F32)
            nc.vector.tensor_mul(tmp2, sb_w1, sb_arr)
            nc.vector.tensor_reduce(
                out=sb_rhs1[:, FT1 + H1:FT1 + 2 * H1],
                in_=tmp2.rearrange("p (h d) -> p h d", h=H1),
                axis=AX.X, op=ALU.add)
            nc.vector.tensor_copy(sb_rhs1[:, 0:FT1], sb_w1)
            nc.vector.tensor_copy(sb_rhs1b, sb_rhs1)

            tmp3 = s0.tile([128, cfg.KT2, C], F32)
            nc.vector.tensor_mul(tmp3, sb_w2r, _bcast(sb_al2, 1, cfg.KT2))
            lr2 = s0.tile([128, cfg.KT2, 2], F32)
            nc.vector.tensor_reduce(
                out=lr2[:, :, 0:1].squeeze(2),
                in_=tmp3, axis=AX.X, op=ALU.add)
            tmp4 = s0.tile([128, cfg.KT2, C], F32)
            nc.vector.tensor_mul(tmp4, sb_w2r, _bcast(sb_ar2, 1, cfg.KT2))
            nc.vector.tensor_reduce(
                out=lr2[:, :, 1:2].squeeze(2),
                in_=tmp4, axis=AX.X, op=ALU.add)
            nc.vector.tensor_copy(sb_rhs2[:, :, C:C + 2], lr2)
            nc.vector.tensor_copy(sb_rhs2[:, :, 0:C], sb_w2r)

            # ---- stage 1: node pass -> t1ft / t1e (replicated) ---------
            NSUB = cfg.NSUB
            n_super = math.ceil(cfg.NT / NSUB)
            with tc.tile_pool(name="ps_node", bufs=2, space="PSUM") as ps_node, \
                 tc.tile_pool(name="xt", bufs=3) as xtp, \
                 tc.tile_pool(name="row1", bufs=3) as rowp:
                for st in range(n_super):
                    t0 = st * NSUB * 128
                    ncols = min(NSUB * 128, N - t0)
                    nsub = math.ceil(ncols / 128)
                    xt_t = xtp.tile([128, NSUB * 128], F32)
                    nc.gpsimd.dma_start(out=xt_t[:, :ncols], in_=xT[:, t0:t0 + ncols])
                    xt_b = xtp.tile([128, NSUB * 128], BF16, tag="xtb")
                    nc.vector.tensor_copy(xt_b[:, :ncols], xt_t[:, :ncols])
                    row_f = rowp.tile([128, NSUB, cfg.ROW1F], BF16)
                    if st % 2 == 0:
                        row_e2 = rowp.tile([128, 2, NSUB, 16], F32, tag="rowe")
                    row_e = row_e2[:, st % 2]
                    ps = ps_node.tile([128, NSUB, 512], F32)
                    for i in range(nsub):
                        nt = min(128, ncols - i * 128)
                        nc.tensor.matmul(ps[:nt, i, 0:W1EXT],
                                         xt_b[:, i * 128:i * 128 + nt],
                                         sb_rhs1b, start=True, stop=True)
                    full = ncols // 128      # rows r = t0 + i*128 + p
                    if full:
                        nc.scalar.copy(
                            row_f[:, 0:full, :], ps[:, 0:full, 0:FT1])
                        nc.vector.tensor_copy(
                            row_e[:, 0:full, :], ps[:, 0:full, FT1:W1EXT])
                        dst_f = bass.AP(
                            tensor=t1ft, offset=t0 * cfg.ROW1F,
                            ap=[[cfg.ROW1F, 128], [128 * cfg.ROW1F, full],
                                [1, cfg.ROW1F]])
                        nc.sync.dma_start(out=dst_f, in_=row_f[:, :full, :])
                        pass
                    if (st % 2 == 1 or st == n_super - 1):
                        st0 = st - (st % 2)
                        nfull = (st - st0) * NSUB + full
                        if nfull:
                            dst_e = bass.AP(
                                tensor=t1e,
                                offset=st0 * NSUB * 128 * cfg.ROW1E,
                                ap=[[cfg.ROW1E, 128],
                                    [128 * cfg.ROW1E, nfull], [1, 16]])
                            nc.sync.dma_start(
                                out=dst_e,
                                in_=row_e2.rearrange(
                                    "p a b c -> p (a b) c")[:, :nfull, :])
                    if full < nsub:          # partial last node tile
                        nt = ncols - full * 128
                        nc.scalar.copy(
                            row_f[:nt, full, :], ps[:nt, full, 0:FT1])
                        nc.vector.tensor_copy(
                            row_e[:nt, full, :], ps[:nt, full, FT1:W1EXT])
                        dst_f = bass.AP(
                            tensor=t1ft, offset=(t0 + full * 128) * cfg.ROW1F,
                            ap=[[cfg.ROW1F, nt], [1, cfg.ROW1F]])
                        nc.sync.dma_start(out=dst_f, in_=row_f[:nt, full, :])
                        dst_e = bass.AP(
                            tensor=t1e, offset=(t0 + full * 128) * cfg.ROW1E,
                            ap=[[cfg.ROW1E, nt], [1, 16]])
                        nc.sync.dma_start(out=dst_e, in_=row_e[:nt, full, :])

            # ---- edge pass pools (shared between the two layers) -------
            gt_p = ctx.enter_context(tc.tile_pool(name="gt", bufs=3))
            gel_p = ctx.enter_context(tc.tile_pool(name="gel", bufs=2))
            ger_p = ctx.enter_context(tc.tile_pool(name="ger", bufs=2))
            e_p = ctx.enter_context(tc.tile_pool(name="eb", bufs=2))
            w_p = ctx.enter_context(tc.tile_pool(name="wb", bufs=2))
            st_p = ctx.enter_context(tc.tile_pool(name="stb", bufs=3))
            stt_p = ctx.enter_context(tc.tile_pool(name="sttb", bufs=2))
            fin_p = ctx.enter_context(tc.tile_pool(name="fin", bufs=2))
            row2_p = ctx.enter_context(tc.tile_pool(name="row2p", bufs=6))

            ps_o = ctx.enter_context(tc.tile_pool(name="ps_o", bufs=2, space="PSUM"))
            ps_t = ctx.enter_context(tc.tile_pool(name="ps_t", bufs=1, space="PSUM"))
            ps_b = ctx.enter_context(tc.tile_pool(name="ps_b", bufs=1, space="PSUM"))
            ps_o2 = ctx.enter_context(tc.tile_pool(name="ps_o2", bufs=1, space="PSUM"))
            ps_st = ctx.enter_context(tc.tile_pool(name="ps_st", bufs=1, space="PSUM"))
            ps_er = ctx.enter_context(tc.tile_pool(name="ps_er", bufs=1, space="PSUM"))

            # ---- stage 2: layer-1 edge pass ----------------------------
            gt = st_b = w_b = None
            ps_acc = None
            for k in range(K_pad):
                g, c = divmod(k, GB)
                if c == 0:
                    gt = gt_p.tile([128, GB, cfg.ROW1F], BF16)
                    gel = gel_p.tile([128, GB, cfg.ROW1E], F32)
                    ger = ger_p.tile([128, GB, cfg.ROW1E], F32)
                    nc.gpsimd.dma_gather(
                        gt, t1ft[:, :], sb_si1[:, g * 128:(g + 1) * 128],
                        cfg.NIDX, nidx_reg, cfg.ROW1F, single_packet=False)
                    nc.gpsimd.dma_gather(
                        gel, t1e[:, :], sb_si1[:, g * 128:(g + 1) * 128],
                        cfg.NIDX, nidx_reg, cfg.ROW1E, single_packet=False)
                    nc.gpsimd.dma_gather(
                        ger, t1e[:, :], sb_di1[:, g * 128:(g + 1) * 128],
                        cfg.NIDX, nidx_reg, cfg.ROW1E, single_packet=False)
                    # e = leaky_relu(el[src] + er[dst]);  ex = exp(e) (dup x2)
                    e_b = e_p.tile([128, GB, H1], BF16, tag="e1a")
                    nc.vector.tensor_add(e_b, gel[:, :, 0:H1],
                                         ger[:, :, H1:2 * H1])
                    e_s = e_p.tile([128, GB, H1], BF16, tag="e1b")
                    nc.vector.tensor_scalar_mul(e_s, e_b, cfg.NEG)
                    e_lr = e_p.tile([128, GB, H1], BF16, tag="e1c")
                    nc.vector.tensor_max(e_lr, e_b, e_s)
                    ex_d = e_p.tile([128, GB, H1, 2], BF16, tag="exd")
                    nc.scalar.activation(ex_d, _bcast(e_lr, 3, 2), ACTF.Exp)
                    # W rows: [ft[src] * ex (per-head) | ex]
                    w_b = w_p.tile([128, GB, FT1 + H1], BF16)
                    nc.vector.tensor_mul(
                        w_b[:, :, 0:FT1].rearrange(
                            "p g (h q t) -> p g h q t", h=H1, t=2),
                        gt.rearrange(
                            "p g (h q t) -> p g h q t", h=H1, t=2),
                        _bcast(ex_d, 3, cfg.D1 // 2))
                    nc.vector.tensor_copy(
                        w_b[:, :, FT1:FT1 + H1], ex_d[:, :, :, 0])
                    # one-hot S for the whole batch (bf16 2x via dup pairs)
                    st_b = st_p.tile([128, GB, 128], BF16, tag="stb")
                    nc.vector.tensor_tensor(
                        st_b.rearrange("p g (q t) -> p g q t", t=2),
                        _bcast(sb_iotab.rearrange("p (q t) -> p q t", t=2),
                               1, GB),
                        _bcast(sb_dlocd[:, g * GB:(g + 1) * GB, :], 2, 64),
                        ALU.is_equal)

                b = int(cb[k])
                if first_of[k]:
                    ps_acc = ps_o.tile([128, FT1 + H1], F32)
                nc.tensor.matmul(ps_acc, st_b[:, c, :], w_b[:, c, :],
                                 start=first_of[k], stop=last_of[k])
                if last_of[k]:
                    s_t = fin_p.tile([128, H1], F32)
                    nc.vector.tensor_scalar_max(s_t, ps_acc[:, FT1:FT1 + H1], 1e-16)
                    rs = fin_p.tile([128, H1], F32)
                    nc.vector.reciprocal(rs, s_t)
                    ht = fin_p.tile([128, FT1], F32)
                    nc.vector.tensor_mul(
                        ht.rearrange("p (h d) -> p h d", h=H1),
                        ps_acc[:, 0:FT1].rearrange("p (h d) -> p h d", h=H1),
                        _bcast(rs, 2, cfg.D1))
                    nc.scalar.activation(sb_h[:, b, :], ht, ACTF.Relu)

                    # ---- layer-2 node prep for this block --------------
                    hT = fin_p.tile([128, cfg.KT2, 128], BF16)
                    for half in range(cfg.KT2):
                        pst = ps_t.tile([128, 128], BF16)
                        nc.tensor.transpose(
                            pst, sb_h[:, b, half * 128:(half + 1) * 128], sb_ident)
                        nc.scalar.copy(hT[:, half, :], pst)
                    ps2 = ps_b.tile([128, C + 2], F32)
                    for half in range(cfg.KT2):
                        nc.tensor.matmul(ps2, hT[:, half, :], sb_rhs2[:, half, :],
                                         start=(half == 0), stop=(half == cfg.KT2 - 1))
                    row2 = row2_p.tile([128, cfg.C2], BF16)
                    nc.scalar.copy(row2, ps2)
                    nc.sync.dma_start(
                        out=t2own_c[b * BLK:(b + 1) * BLK, :], in_=row2)
                    # own-block er2 column, [vloc, blk] layout for the
                    # on-chip dst broadcast in stage 3
                    q2_ap = bass.AP(
                        tensor=t2own_c, offset=b * BLK * cfg.C2 + cfg.ER2,
                        ap=[[cfg.C2, 128], [1, 1]])
                    nc.sync.dma_start(out=sb_q2[:, b, :], in_=q2_ap)

            # ---- er2[dst] broadcast prep: gate on the last q2 write so
            # the scheduler runs it inside the collective window (where all
            # engines but Pool idle) instead of hoisting it into stage 1 ----
            n_b2 = K_pad // cfg.GB2
            sb_er2 = consts.tile([128, n_b2, cfg.GB2, 1], F32)
            sb_st2 = consts.tile([128, K_pad, 128], BF16)
            zq = e_p.tile([128, 1], BF16, tag="zq")
            nc.vector.tensor_scalar_mul(zq, sb_q2[:, 18, :], 0.0)
            sb_iotab2 = consts.tile([128, 128], BF16)
            nc.vector.tensor_add(sb_iotab2, sb_iotab,
                                 _bcast(zq.squeeze(1), 1, 128))
            for g in range(n_b2):
                st2p = sb_st2[:, g * cfg.GB2:(g + 1) * cfg.GB2, :]
                nc.vector.tensor_tensor(
                    st2p.rearrange("p g (q t) -> p g q t", t=2),
                    _bcast(sb_iotab2.rearrange("p (q t) -> p q t", t=2),
                           1, cfg.GB2),
                    _bcast(sb_dlocd[:, g * cfg.GB2:(g + 1) * cfg.GB2, :], 2, 64),
                    ALU.is_equal)
                stT_ps = ps_st.tile([128, cfg.GB2, 128], BF16)
                for c16 in range(cfg.GB2):
                    nc.tensor.transpose(
                        stT_ps[:, c16, :], st2p[:, c16, :], sb_ident)
                stT_sb = stt_p.tile([128, cfg.GB2, 128], BF16)
                if g % 2 == 0:
                    nc.scalar.copy(stT_sb, stT_ps)
                else:
                    nc.vector.tensor_copy(stT_sb, stT_ps)
                er2_ps = ps_er.tile([128, cfg.GB2, 1], F32)
                for c16 in range(cfg.GB2):
                    bb = int(cb[g * cfg.GB2 + c16])
                    nc.tensor.matmul(er2_ps[:, c16, :], stT_sb[:, c16, :],
                                     sb_q2[:, bb, :], start=True, stop=True)
                nc.vector.tensor_copy(sb_er2[:, g, :, :], er2_ps)

            nc.gpsimd.collective_compute(
                "AllGather", ALU.bypass,
                replica_groups=[list(range(cfg.NCORES))],
                ins=[t2own_c[:, :].opt()],
                outs=[table2c[:, :].opt()])
            nc.sync.dma_start(out=table2[:, 0:cfg.C2], in_=table2c[:, :])

            # ---- stage 3: layer-2 edge pass ----------------------------
            g2 = st2_b = w2_b = None
            ps_acc2 = None
            GB2 = cfg.GB2
            for k in range(K_pad):
                g, c = divmod(k, GB2)
                if c == 0:
                    icols = cfg.NIDX2 // 16
                    g2 = gt_p.tile([128, GB2, cfg.ROW2], BF16, tag="gt")
                    nc.gpsimd.dma_gather(
                        g2, table2[:, :], sb_si2[:, g * icols:(g + 1) * icols],
                        cfg.NIDX2, nidx2_reg, cfg.ROW2, single_packet=False)
                    st2_b = sb_st2[:, g * GB2:(g + 1) * GB2, :]
                    e2_b = e_p.tile([128, GB2, 1], BF16, tag="e2a")
                    nc.vector.tensor_add(e2_b, g2[:, :, cfg.EL2:cfg.EL2 + 1],
                                         sb_er2[:, g, :, :])
                    e2_s = e_p.tile([128, GB2, 1], BF16, tag="e2b")
                    nc.vector.tensor_scalar_mul(e2_s, e2_b, cfg.NEG)
                    e2_lr = e_p.tile([128, GB2, 1], BF16, tag="e2c")
                    nc.vector.tensor_max(e2_lr, e2_b, e2_s)
                    ex2_d = e_p.tile([128, GB2, 2], BF16, tag="ex2")
                    nc.scalar.activation(
                        ex2_d, _bcast(e2_lr.squeeze(2), 2, 2), ACTF.Exp)
                    w2_b = w_p.tile([128, GB2, C + 1], BF16, tag="wb2")
                    nc.vector.tensor_mul(
                        w2_b[:, :, 0:C].rearrange("p g (q t) -> p g q t", t=2),
                        g2[:, :, 0:C].rearrange("p g (q t) -> p g q t", t=2),
                        _bcast(ex2_d, 2, C // 2))
                    nc.vector.tensor_copy(
                        w2_b[:, :, C:C + 1], ex2_d[:, :, 0:1])

                b = int(cb[k])
                if first_of[k]:
                    ps_acc2 = ps_o2.tile([128, C + 1], F32)
                nc.tensor.matmul(ps_acc2, st2_b[:, c, :], w2_b[:, c, :],
                                 start=first_of[k], stop=last_of[k])
                if last_of[k]:
                    s2_t = fin_p.tile([128, 1], F32, tag="fin2")
                    nc.vector.tensor_scalar_max(s2_t, ps_acc2[:, C:C + 1], 1e-16)
                    rs2 = fin_p.tile([128, 1], F32, tag="fin2")
                    nc.vector.reciprocal(rs2, s2_t)
                    ot = fin_p.tile([128, C], F32, tag="fin2")
                    nc.scalar.activation(ot, ps_acc2[:, 0:C], ACTF.Copy,
                                         scale=rs2)
                    nc.sync.dma_start(out=out[b * BLK:(b + 1) * BLK, :], in_=ot)

    # Bacc.compile() legalizes waits (<=1 per inst), inserts library loads,
    # and packs extended-ISA instruction bytes.
    nc.finalize()
    return nc


# ---------------------------------------------------------------- driver

def _make_in_maps(cfg, inputs, prep):
    x = np.asarray(inputs["x"], np.float32)
    W1 = np.ascontiguousarray(np.asarray(inputs["W1"], np.float32))
    attn_l1 = np.asarray(inputs["attn_l1"], np.float32)
    attn_r1 = np.asarray(inputs["attn_r1"], np.float32)
    W2 = np.asarray(inputs["W2"], np.float32)
    attn_l2 = np.asarray(inputs["attn_l2"], np.float32)
    attn_r2 = np.asarray(inputs["attn_r2"], np.float32)
    import ml_dtypes

    iota = np.tile(np.arange(128, dtype=np.float32)[None, :], (128, 1))
    common = {
        "xT": np.ascontiguousarray(x.T[:, prep["order"]]),
        "W1": W1,
        "al_rep": np.ascontiguousarray(
            np.tile(attn_l1.reshape(1, -1), (128, 1))),
        "ar_rep": np.ascontiguousarray(
            np.tile(attn_r1.reshape(1, -1), (128, 1))),
        "W2r": np.ascontiguousarray(
            W2.reshape(cfg.KT2, 128, cfg.C).transpose(1, 0, 2).reshape(
                128, cfg.KT2 * cfg.C)),
        "al2_rep": np.ascontiguousarray(np.tile(attn_l2.reshape(1, -1), (128, 1))),
        "ar2_rep": np.ascontiguousarray(np.tile(attn_r2.reshape(1, -1), (128, 1))),
        "iotab": np.ascontiguousarray(iota.astype(ml_dtypes.bfloat16)),
    }
    in_maps = []
    for c in range(cfg.NCORES):
        m = dict(common)
        m["src_idx1"] = prep["src_idx1"][c]
        m["dst_idx1"] = prep["dst_idx1"][c]
        m["src_idx2"] = prep["src_idx2"][c]
        m["dlocd"] = prep["dlocd"][c]
        in_maps.append(m)
    return in_maps


def build_all(inputs, cfg=CFG):
    prep = _host_prep(cfg, inputs["src"], inputs["dst"])
    nc = _build(cfg, prep["K_pad"], prep["cb"])
    in_maps = _make_in_maps(cfg, inputs, prep)
    return nc, in_maps, prep["order"]


def kernel(**inputs):
    cfg = CFG
    nc, in_maps, order = build_all(inputs, cfg)
    from concourse.bass_utils import run_bass_kernel_spmd
    res = run_bass_kernel_spmd(nc, in_maps, core_ids=list(range(cfg.NCORES)))
    rows = np.concatenate(
        [res.results[c]["out"][:cfg.NB] for c in range(cfg.NCORES)], axis=0)
    out = np.empty_like(rows)
    out[order] = rows
    return np.ascontiguousarray(out, dtype=np.float32)
